# revision 27
# baseline (speedup 1.0000x reference)
"""Distributed Bass attention kernel for 8 TRN2 NeuronCores.

Problem: single-head causal attention, B=4, S=2048, d_model=1024, d_head=64.
  q = x@WQ.T+bq; k = x@WK.T+bk; v = x@WV.T+bv (v is d_model wide)
  out = softmax(causal(q@k.T)) @ v

Sharding: core = 2*b + half. Each core computes batch b, output channels
[half*512, (half+1)*512). Q/K/scores/softmax are duplicated within a batch
pair (cheap); V projection and attn@V are channel-split. No collectives.

Layout tricks:
  - x fed pre-transposed and bf16 (xT [d, S]) so projections contract
    d_model on partitions; q/k projections accumulate in fp32 PSUM with the
    bias folded in via a K=1 ones matmul.
  - scores computed transposed [keys, queries] so attn@V uses the exp'd P
    tiles directly as the stationary operand - no transposes anywhere.
  - scores matmul pads the 64-wide head dim to K=128 and uses the spare
    array rows for extra precision: lhsT = [k_hi; k_lo] (bf16 split) against
    rhs = [q_hi; q_hi] computes (k_hi + k_lo) . q_hi in one full-array pass.
  - softmax without max-subtraction (|logits| <= ~50 => exp fits fp32 fine).
    P stays unnormalized on-chip; per-query key-sums are accumulated as two
    parallel tile-sum chains (DVE + GpSimd), cast to bf16 and exported raw -
    the host does the final partition sum and the divide in fp32.
  - streaming front end: input DMAs interleaved (xt chunk c, wv chunk c) and
    the QK projection + first V-projection group consume each chunk as it
    lands, so the PE never waits for the full x to arrive. Warmup matmuls +
    the bias starter matmuls cover the initial DMA window (and release the
    HAM clock-gate).
  - causal masks DMA'd on the slow software queue (plenty of slack) so the
    fast queue carries only the latency-critical x/WV stream.
  - outputs exported bf16 (numerators and raw key-sums); host divides in
    fp32, so bf16 rounding is a ~0.4% incoherent noise, well inside budget.
  - attention blocks processed in reverse size order and software-pipelined:
    block j's attn@V matmuls are interleaved into block j-1's scores/exp
    emission so the in-order PE queue never stalls.
"""

import sys

if "/opt/trn_rl_repo" not in sys.path:
    sys.path.insert(0, "/opt/trn_rl_repo")

import numpy as np

from concourse import bacc, tile, mybir
import concourse.bass as bass
from concourse.bass_utils import run_bass_kernel_spmd

B, S, D, HD = 4, 2048, 1024, 64
N_CORES = 8
CPC = 512  # output channels per core
NCHUNK = 8  # d_model / 128

f32 = mybir.dt.float32
bf16 = mybir.dt.bfloat16
AF = mybir.ActivationFunctionType
ALU = mybir.AluOpType

_cache = {}


def _build():
    nc = bacc.Bacc("TRN2", target_bir_lowering=False, debug=False, num_devices=N_CORES)

    xT = nc.dram_tensor("xT", [NCHUNK, 128, S], bf16, kind="ExternalInput")
    wqkT = nc.dram_tensor("wqkT", [128, NCHUNK, 128], bf16, kind="ExternalInput")
    bqkr = nc.dram_tensor("bqkr", [1, 128], bf16, kind="ExternalInput")
    wvT = nc.dram_tensor("wvT", [NCHUNK, 128, CPC], bf16, kind="ExternalInput")
    masks = nc.dram_tensor("masks", [128, 4, 512], bf16, kind="ExternalInput")
    # outputs are written in pairs ([128, 1024] = 2KB DMA lines, which the
    # DMA engines move at ~2x the packet efficiency of 1KB lines)
    out = nc.dram_tensor("out", [8, 128, 2 * CPC], bf16, kind="ExternalOutput")
    rsum = nc.dram_tensor("rsum", [4, 128, 2, 512], bf16, kind="ExternalOutput")

    with tile.TileContext(nc) as tc:
        with (
            tc.tile_pool(name="big", bufs=1) as big,
            tc.tile_pool(name="ppool", bufs=30) as ppool,
            tc.tile_pool(name="opool", bufs=4) as opool,
            tc.tile_pool(name="small", bufs=4) as small,
            tc.tile_pool(name="spool", bufs=4) as spool,
            tc.tile_pool(name="ps_a", bufs=4, space=bass.MemorySpace.PSUM) as ps_a,
            tc.tile_pool(name="ps_b", bufs=4, space=bass.MemorySpace.PSUM) as ps_b,
        ):
            # persistent SBUF tiles
            xt = big.tile([128, NCHUNK, S], bf16, tag="xt")  # 32KB/p
            wqk = big.tile([128, NCHUNK, 128], bf16, tag="wqk")  # 2KB/p
            wv = big.tile([128, NCHUNK, CPC], bf16, tag="wv")  # 8KB/p
            bqk_sb = big.tile([1, 128], bf16, tag="bqk")
            mask_sb = big.tile([128, 4, 512], bf16, tag="mask")  # 4KB/p
            qkhi = big.tile([128, S], bf16, tag="qkhi")  # rows q_hi / k_hi
            khiklo = big.tile([128, S], bf16, tag="khiklo")  # [k_hi; k_lo]
            qhiqhi = big.tile([128, S], bf16, tag="qhiqhi")  # [q_hi; q_hi]
            v_sb = big.tile([128, 16, CPC], bf16, tag="v")  # 16KB/p
            ones_b = big.tile([128, 512], bf16, tag="ones_b")

            # input DMAs: the tiny bias row first (unblocks the bias starter
            # matmuls), then wqk, then xt/wv chunk-interleaved so the stream
            # loop's per-chunk consumers are fed in order.
            nc.sync.dma_start(out=bqk_sb[:, :], in_=bqkr[:, :])
            nc.sync.dma_start(out=wqk[:, :, :], in_=wqkT[:, :, :])
            for c in range(NCHUNK):
                nc.sync.dma_start(out=xt[:, c, :], in_=xT[c, :, :])
                nc.sync.dma_start(out=wv[:, c, :], in_=wvT[c, :, :])
            # masks last: not needed until block 3's diagonal score tiles
            # (~30us in), so they must not compete with the x/WV stream
            nc.sync.dma_start(out=mask_sb[:, :, :], in_=masks[:, :, :])
            nc.vector.memset(ones_b[:, :], 1.0)

            # PE warmup: dummy matmuls on the ones tile while input DMA
            # streams, so the HAM clock-gate is released before real work.
            # Chained into out[0] (overwritten later) so DCE keeps them.
            warm_ps = ps_b.tile([128, 512], f32, tag="vps", name="warm_ps")
            for w in range(6):
                nc.tensor.matmul(
                    warm_ps[:, :],
                    ones_b[:, 0:128],
                    ones_b[:, 0:512],
                    start=(w == 0),
                    stop=(w == 5),
                )
            warm_sb = small.tile([128, 512], bf16, tag="warm", name="warm_sb")
            nc.vector.tensor_copy(warm_sb[:, :], warm_ps[:, :])
            nc.sync.dma_start(out=out[0, :, 0:512], in_=warm_sb[:, :])

            # ---- streamed QK projection + V group 0 ----
            # qkT [128h (64 q + 64 k), S]; bias starter matmuls only need
            # bqk_sb + ones, so they run during the DMA window.
            qk_ps = [
                ps_a.tile([128, 512], f32, tag="scps", name=f"qkps{j}")
                for j in range(4)
            ]
            for j in range(4):
                nc.tensor.matmul(
                    qk_ps[j][:, :],
                    bqk_sb[:, :],
                    ones_b[0:1, 0:512],
                    start=True,
                    stop=False,
                )
            v_ps0 = [
                ps_b.tile([128, CPC], f32, tag="vps", name=f"vps{t}")
                for t in range(4)
            ]
            for c in range(NCHUNK):
                for j in range(4):
                    nc.tensor.matmul(
                        qk_ps[j][:, :],
                        wqk[:, c, :],
                        xt[:, c, 512 * j : 512 * (j + 1)],
                        start=False,
                        stop=(c == NCHUNK - 1),
                    )
                for t in range(4):
                    nc.tensor.matmul(
                        v_ps0[t][:, :],
                        xt[:, c, 128 * t : 128 * (t + 1)],
                        wv[:, c, :],
                        start=(c == 0),
                        stop=(c == NCHUNK - 1),
                    )

            # ---- qk evac: hi/lo split + packing ----
            for j in range(4):
                blk = slice(512 * j, 512 * (j + 1))
                nc.scalar.copy(qkhi[:, blk], qk_ps[j][:, :])
                # k_lo = (k + bias) - k_hi, straight into khiklo rows 64+
                nc.vector.tensor_tensor(
                    khiklo[64:128, blk],
                    qk_ps[j][64:128, :],
                    qkhi[64:128, blk],
                    ALU.subtract,
                )
                nc.sync.dma_start(out=khiklo[0:64, blk], in_=qkhi[64:128, blk])
                nc.sync.dma_start(out=qhiqhi[0:64, blk], in_=qkhi[0:64, blk])
                nc.sync.dma_start(out=qhiqhi[64:128, blk], in_=qkhi[0:64, blk])
            for t in range(4):
                if t % 2 == 1:
                    nc.vector.tensor_copy(v_sb[:, t, :], v_ps0[t][:, :])
                else:
                    nc.scalar.copy(v_sb[:, t, :], v_ps0[t][:, :])

            # ---- attention helpers ----
            def emit_scores(j, i, Ssum):
                # K=128 single matmul: rows 0-63 k_hi x q_hi, rows 64-127
                # k_lo x q_hi => scores = (k_hi + k_lo) . q_hi
                sc_ps = ps_a.tile([128, 512], f32, tag="scps", name=f"scps{j}_{i}")
                nc.tensor.matmul(
                    sc_ps[:, :],
                    khiklo[:, 128 * i : 128 * (i + 1)],
                    qhiqhi[:, 512 * j : 512 * (j + 1)],
                    start=True,
                    stop=True,
                )
                p = ppool.tile([128, 512], bf16, tag="p", name=f"p{j}_{i}")
                nc.scalar.activation(p[:, :], sc_ps[:, :], AF.Exp)
                if i >= 4 * j:
                    nc.vector.tensor_tensor(
                        p[:, :], p[:, :], mask_sb[:, i - 4 * j, :], ALU.mult
                    )
                eng = nc.vector if i % 2 == 0 else nc.gpsimd
                Sc = Ssum[i % 2]
                if i < 2:
                    eng.tensor_copy(Sc[:, :], p[:, :])
                else:
                    eng.tensor_tensor(Sc[:, :], Sc[:, :], p[:, :], ALU.add)
                return [p]

            def export_rsum(j, Ssum):
                # cast the fp32 chains to bf16 (DVE) and export as one
                # [128, 1024] DMA on the fast queue
                rs_bf = small.tile([128, 2, 512], bf16, tag="rsbf", name=f"rsbf{j}")
                for c in range(2):
                    nc.vector.tensor_copy(rs_bf[:, c, :], Ssum[c][:, :])
                nc.sync.dma_start(out=rsum[j, :, :, :], in_=rs_bf[:, :, :])

            def attnv_ops(j, P, reverse=False):
                ops = []
                for tq in ([3, 2, 1, 0] if reverse else range(4)):
                    t = 4 * j + tq
                    ops.append(("alloc", t))
                    for i in range(t + 1):
                        ops.append(("mm", t, i))
                    ops.append(("evac", t))
                return ops

            def emit_attnv_op(op, P, state):
                if op[0] == "alloc":
                    t = op[1]
                    state[t] = ps_b.tile([128, CPC], f32, tag="vps", name=f"ops{t}")
                elif op[0] == "mm":
                    _, t, i = op
                    nc.tensor.matmul(
                        state[t][:, :],
                        P[i][:, 128 * (t % 4) : 128 * (t % 4) + 128],
                        v_sb[:, i, :],
                        start=(i == 0),
                        stop=(i == t),
                    )
                else:
                    t = op[1]
                    pair = t // 2
                    key = ("stage", pair)
                    if key not in state:
                        state[key] = opool.tile(
                            [128, 2, 512], bf16, tag="osb", name=f"osb{pair}"
                        )
                    st = state[key]
                    nc.scalar.copy(st[:, t % 2, :], state[t][:, :])
                    nfill = state.get(("nfill", pair), 0) + 1
                    state[("nfill", pair)] = nfill
                    if nfill == 2:
                        nc.sync.dma_start(out=out[pair, :, :], in_=st[:, :, :])

            # ---- V projection groups 1-3, with block 3's scores interleaved
            # into the tail so its exp chain (ACT) finishes before attn@V
            # needs P ----
            Ssum3 = [
                spool.tile([128, 512], f32, tag=f"S{c}", name=f"S3_{c}")
                for c in range(2)
            ]
            P3 = []
            for t in range(4, 16):
                v_ps = ps_b.tile([128, CPC], f32, tag="vps")
                for c in range(NCHUNK):
                    nc.tensor.matmul(
                        v_ps[:, :],
                        xt[:, c, 128 * t : 128 * (t + 1)],
                        wv[:, c, :],
                        start=(c == 0),
                        stop=(c == NCHUNK - 1),
                    )
                if t % 2 == 1:
                    nc.vector.tensor_copy(v_sb[:, t, :], v_ps[:, :])
                else:
                    nc.scalar.copy(v_sb[:, t, :], v_ps[:, :])
                if 8 <= t < 16:
                    P3.extend(emit_scores(3, 2 * (t - 8), Ssum3))
                    P3.extend(emit_scores(3, 2 * (t - 8) + 1, Ssum3))
            export_rsum(3, Ssum3)

            # ---- attention: blocks in reverse order, software-pipelined ----
            # Section s runs block j's scores/exp/rowsum while the previous
            # section's (larger) block does its attn@V - interleaved in PE
            # program order so neither phase stalls the in-order PE queue.
            prev = (3, P3)  # block 3 scored during vproj; attn@V pending
            for j in [2, 1, 0, None]:
                av = attnv_ops(*prev, reverse=(j is None)) if prev is not None else []
                avP = prev[1] if prev is not None else None
                av_state = {}
                if j is None:
                    for op in av:
                        emit_attnv_op(op, avP, av_state)
                    break
                n = 4 * j + 4
                Ssum = [
                    spool.tile([128, 512], f32, tag=f"S{c}", name=f"S{j}_{c}")
                    for c in range(2)
                ]
                P = []
                A = list(range(n))  # score emissions
                # front-load a couple of score pairs, then interleave the
                # previous block's attn@V ops
                front = min(3, len(A))
                k_av = 0
                for idx, i in enumerate(A):
                    P.extend(emit_scores(j, i, Ssum))
                    if idx >= front - 1:
                        want = (idx + 1 - front + 1) * len(av) / max(
                            1, len(A) - front + 1
                        )
                        while k_av < len(av) and k_av < want:
                            emit_attnv_op(av[k_av], avP, av_state)
                            k_av += 1
                while k_av < len(av):
                    emit_attnv_op(av[k_av], avP, av_state)
                    k_av += 1
                export_rsum(j, Ssum)
                prev = (j, P)

    nc.compile()
    return nc


def _get_nc():
    if "nc" not in _cache:
        _cache["nc"] = _build()
    return _cache["nc"]


def _prep_in_maps(x, WQ_w, WQ_b, WK_w, WK_b, WV_w, WV_b):
    bf = mybir.dt.np(bf16)
    wqk = np.concatenate([WQ_w, WK_w], axis=0)  # [128, D]
    wqkT = np.ascontiguousarray(
        wqk.T.reshape(NCHUNK, 128, 128).transpose(1, 0, 2)
    ).astype(bf)
    bqkr = np.concatenate([WQ_b, WK_b]).reshape(1, 128).astype(bf)

    # masks[kk, m, qq] = 1 if 128*m + kk <= qq else 0
    kk = np.arange(128)[:, None]
    qq = np.arange(512)[None, :]
    masks = np.ascontiguousarray(
        np.stack([(128 * m + kk <= qq) for m in range(4)], axis=0).transpose(1, 0, 2)
    ).astype(bf)
    in_maps = []
    for core in range(N_CORES):
        b, half = core // 2, core % 2
        xTb = np.ascontiguousarray(x[b].T).reshape(NCHUNK, 128, S).astype(bf)
        wv_sl = WV_w[half * CPC : (half + 1) * CPC]  # [CPC, D]
        wvT = np.ascontiguousarray(wv_sl.T).reshape(NCHUNK, 128, CPC).astype(bf)
        in_maps.append(
            {
                "xT": xTb,
                "wqkT": wqkT,
                "bqkr": bqkr,
                "wvT": wvT,
                "masks": masks,
            }
        )
    return in_maps


def _run(in_maps, trace=False, **kw):
    nc = _get_nc()
    return run_bass_kernel_spmd(
        nc, in_maps, core_ids=list(range(N_CORES)), trace=trace, **kw
    )


def kernel(x, WQ_w, WQ_b, WK_w, WK_b, WV_w, WV_b):
    x = np.asarray(x, dtype=np.float32)
    in_maps = _prep_in_maps(
        x,
        np.asarray(WQ_w, np.float32),
        np.asarray(WQ_b, np.float32),
        np.asarray(WK_w, np.float32),
        np.asarray(WK_b, np.float32),
        np.asarray(WV_w, np.float32),
        np.asarray(WV_b, np.float32),
    )
    res = _run(in_maps, trace=False)
    out = np.empty((B, S, D), dtype=np.float32)
    for core in range(N_CORES):
        b, half = core // 2, core % 2
        shard = (
            res.results[core]["out"]
            .astype(np.float32)
            .reshape(8, 128, 2, 512)
            .transpose(0, 2, 1, 3)
            .reshape(S, CPC)
        )
        if half == 0:
            rs = (
                res.results[core]["rsum"]
                .astype(np.float32)
                .sum(axis=(1, 2))
                .reshape(S)
            )
            out[b] = 0.0
        out[b, :, half * CPC : (half + 1) * CPC] = shard
        if half == 1:
            out[b] /= rs[:, None]
    out += np.asarray(WV_b, np.float32)[None, None, :]
    return out


# revision 28
# speedup vs baseline: 1.0084x; 1.0084x over previous
"""Distributed Bass attention kernel for 8 TRN2 NeuronCores.

Problem: single-head causal attention, B=4, S=2048, d_model=1024, d_head=64.
  q = x@WQ.T+bq; k = x@WK.T+bk; v = x@WV.T+bv (v is d_model wide)
  out = softmax(causal(q@k.T)) @ v

Sharding: core = 2*b + half. Each core computes batch b, output channels
[half*512, (half+1)*512). Q/K/scores/softmax are duplicated within a batch
pair (cheap); V projection and attn@V are channel-split. No collectives.

Layout tricks:
  - x fed pre-transposed and bf16 (xT [d, S]) so projections contract
    d_model on partitions; q/k projections accumulate in fp32 PSUM with the
    bias folded in via a K=1 ones matmul.
  - scores computed transposed [keys, queries] so attn@V uses the exp'd P
    tiles directly as the stationary operand - no transposes anywhere.
  - scores matmul pads the 64-wide head dim to K=128 and uses the spare
    array rows for extra precision: lhsT = [k_hi; k_lo] (bf16 split) against
    rhs = [q_hi; q_hi] computes (k_hi + k_lo) . q_hi in one full-array pass.
  - softmax without max-subtraction (|logits| <= ~50 => exp fits fp32 fine).
    P stays unnormalized on-chip; per-query key-sums are accumulated as two
    parallel tile-sum chains (DVE + GpSimd), cast to bf16 and exported raw -
    the host does the final partition sum and the divide in fp32.
  - streaming front end: input DMAs interleaved (xt chunk c, wv chunk c) and
    the QK projection + first V-projection group consume each chunk as it
    lands, so the PE never waits for the full x to arrive. Warmup matmuls +
    the bias starter matmuls cover the initial DMA window (and release the
    HAM clock-gate).
  - causal masks DMA'd on the slow software queue (plenty of slack) so the
    fast queue carries only the latency-critical x/WV stream.
  - outputs exported bf16 (numerators and raw key-sums); host divides in
    fp32, so bf16 rounding is a ~0.4% incoherent noise, well inside budget.
  - attention blocks processed in reverse size order and software-pipelined:
    block j's attn@V matmuls are interleaved into block j-1's scores/exp
    emission so the in-order PE queue never stalls.
"""

import sys

if "/opt/trn_rl_repo" not in sys.path:
    sys.path.insert(0, "/opt/trn_rl_repo")

import numpy as np

from concourse import bacc, tile, mybir
import concourse.bass as bass
from concourse.bass_utils import run_bass_kernel_spmd

B, S, D, HD = 4, 2048, 1024, 64
N_CORES = 8
CPC = 512  # output channels per core
NCHUNK = 8  # d_model / 128

f32 = mybir.dt.float32
bf16 = mybir.dt.bfloat16
AF = mybir.ActivationFunctionType
ALU = mybir.AluOpType

_cache = {}


def _build():
    nc = bacc.Bacc("TRN2", target_bir_lowering=False, debug=False, num_devices=N_CORES)

    xT = nc.dram_tensor("xT", [NCHUNK, 128, S], bf16, kind="ExternalInput")
    wqkT = nc.dram_tensor("wqkT", [128, NCHUNK, 128], bf16, kind="ExternalInput")
    bqkr = nc.dram_tensor("bqkr", [1, 128], bf16, kind="ExternalInput")
    wvT = nc.dram_tensor("wvT", [NCHUNK, 128, CPC], bf16, kind="ExternalInput")
    masks = nc.dram_tensor("masks", [128, 4, 512], bf16, kind="ExternalInput")
    # outputs are written in pairs ([128, 1024] = 2KB DMA lines, which the
    # DMA engines move at ~2x the packet efficiency of 1KB lines)
    out = nc.dram_tensor("out", [8, 128, 2 * CPC], bf16, kind="ExternalOutput")
    rsum = nc.dram_tensor("rsum", [4, 128, 2, 512], bf16, kind="ExternalOutput")

    with tile.TileContext(nc) as tc:
        with (
            tc.tile_pool(name="big", bufs=1) as big,
            tc.tile_pool(name="ppool", bufs=30) as ppool,
            tc.tile_pool(name="opool", bufs=4) as opool,
            tc.tile_pool(name="small", bufs=4) as small,
            tc.tile_pool(name="spool", bufs=4) as spool,
            tc.tile_pool(name="ps_a", bufs=4, space=bass.MemorySpace.PSUM) as ps_a,
            tc.tile_pool(name="ps_b", bufs=4, space=bass.MemorySpace.PSUM) as ps_b,
        ):
            # persistent SBUF tiles
            xt = big.tile([128, NCHUNK, S], bf16, tag="xt")  # 32KB/p
            wqk = big.tile([128, NCHUNK, 128], bf16, tag="wqk")  # 2KB/p
            wv = big.tile([128, NCHUNK, CPC], bf16, tag="wv")  # 8KB/p
            bqk_sb = big.tile([1, 128], bf16, tag="bqk")
            mask_sb = big.tile([128, 4, 512], bf16, tag="mask")  # 4KB/p
            qkhi = big.tile([128, S], bf16, tag="qkhi")  # rows q_hi / k_hi
            khiklo = big.tile([128, S], bf16, tag="khiklo")  # [k_hi; k_lo]
            qhiqhi = big.tile([128, S], bf16, tag="qhiqhi")  # [q_hi; q_hi]
            v_sb = big.tile([128, 16, CPC], bf16, tag="v")  # 16KB/p
            ones_b = big.tile([128, 512], bf16, tag="ones_b")

            # input DMAs: the tiny bias row first (unblocks the bias starter
            # matmuls), then wqk, then xt/wv chunk-interleaved so the stream
            # loop's per-chunk consumers are fed in order.
            nc.sync.dma_start(out=bqk_sb[:, :], in_=bqkr[:, :])
            nc.sync.dma_start(out=wqk[:, :, :], in_=wqkT[:, :, :])
            for c in range(NCHUNK):
                nc.sync.dma_start(out=xt[:, c, :], in_=xT[c, :, :])
                nc.sync.dma_start(out=wv[:, c, :], in_=wvT[c, :, :])
            # masks last: not needed until block 3's diagonal score tiles
            # (~30us in), so they must not compete with the x/WV stream
            nc.sync.dma_start(out=mask_sb[:, :, :], in_=masks[:, :, :])
            nc.vector.memset(ones_b[:, :], 1.0)

            # PE warmup: dummy matmuls on the ones tile while input DMA
            # streams, so the HAM clock-gate is released before real work.
            # Chained into out[0] (overwritten later) so DCE keeps them.
            warm_ps = ps_b.tile([128, 512], f32, tag="vps", name="warm_ps")
            for w in range(10):
                nc.tensor.matmul(
                    warm_ps[:, :],
                    ones_b[:, 0:128],
                    ones_b[:, 0:512],
                    start=(w == 0),
                    stop=(w == 9),
                )
            warm_sb = small.tile([128, 512], bf16, tag="warm", name="warm_sb")
            nc.vector.tensor_copy(warm_sb[:, :], warm_ps[:, :])
            nc.sync.dma_start(out=out[0, :, 0:512], in_=warm_sb[:, :])

            # ---- streamed QK projection + V group 0 ----
            # qkT [128h (64 q + 64 k), S]; bias starter matmuls only need
            # bqk_sb + ones, so they run during the DMA window.
            qk_ps = [
                ps_a.tile([128, 512], f32, tag="scps", name=f"qkps{j}")
                for j in range(4)
            ]
            for j in range(4):
                nc.tensor.matmul(
                    qk_ps[j][:, :],
                    bqk_sb[:, :],
                    ones_b[0:1, 0:512],
                    start=True,
                    stop=False,
                )
            v_ps0 = [
                ps_b.tile([128, CPC], f32, tag="vps", name=f"vps{t}")
                for t in range(4)
            ]
            for c in range(NCHUNK):
                for j in range(4):
                    nc.tensor.matmul(
                        qk_ps[j][:, :],
                        wqk[:, c, :],
                        xt[:, c, 512 * j : 512 * (j + 1)],
                        start=False,
                        stop=(c == NCHUNK - 1),
                    )
                for t in range(4):
                    nc.tensor.matmul(
                        v_ps0[t][:, :],
                        xt[:, c, 128 * t : 128 * (t + 1)],
                        wv[:, c, :],
                        start=(c == 0),
                        stop=(c == NCHUNK - 1),
                    )

            # ---- qk evac: hi/lo split + packing ----
            for j in range(4):
                blk = slice(512 * j, 512 * (j + 1))
                nc.scalar.copy(qkhi[:, blk], qk_ps[j][:, :])
                # k_lo = (k + bias) - k_hi, straight into khiklo rows 64+
                nc.vector.tensor_tensor(
                    khiklo[64:128, blk],
                    qk_ps[j][64:128, :],
                    qkhi[64:128, blk],
                    ALU.subtract,
                )
                nc.sync.dma_start(out=khiklo[0:64, blk], in_=qkhi[64:128, blk])
                nc.sync.dma_start(out=qhiqhi[0:64, blk], in_=qkhi[0:64, blk])
                nc.sync.dma_start(out=qhiqhi[64:128, blk], in_=qkhi[0:64, blk])
            for t in range(4):
                if t % 2 == 1:
                    nc.vector.tensor_copy(v_sb[:, t, :], v_ps0[t][:, :])
                else:
                    nc.scalar.copy(v_sb[:, t, :], v_ps0[t][:, :])

            # ---- attention helpers ----
            def emit_scores(j, i, Ssum):
                # K=128 single matmul: rows 0-63 k_hi x q_hi, rows 64-127
                # k_lo x q_hi => scores = (k_hi + k_lo) . q_hi
                sc_ps = ps_a.tile([128, 512], f32, tag="scps", name=f"scps{j}_{i}")
                nc.tensor.matmul(
                    sc_ps[:, :],
                    khiklo[:, 128 * i : 128 * (i + 1)],
                    qhiqhi[:, 512 * j : 512 * (j + 1)],
                    start=True,
                    stop=True,
                )
                p = ppool.tile([128, 512], bf16, tag="p", name=f"p{j}_{i}")
                nc.scalar.activation(p[:, :], sc_ps[:, :], AF.Exp)
                if i >= 4 * j:
                    nc.vector.tensor_tensor(
                        p[:, :], p[:, :], mask_sb[:, i - 4 * j, :], ALU.mult
                    )
                eng = nc.vector if i % 2 == 0 else nc.gpsimd
                Sc = Ssum[i % 2]
                if i < 2:
                    eng.tensor_copy(Sc[:, :], p[:, :])
                else:
                    eng.tensor_tensor(Sc[:, :], Sc[:, :], p[:, :], ALU.add)
                return [p]

            def export_rsum(j, Ssum):
                # cast the fp32 chains to bf16 (DVE) and export as one
                # [128, 1024] DMA on the fast queue
                rs_bf = small.tile([128, 2, 512], bf16, tag="rsbf", name=f"rsbf{j}")
                for c in range(2):
                    nc.vector.tensor_copy(rs_bf[:, c, :], Ssum[c][:, :])
                nc.sync.dma_start(out=rsum[j, :, :, :], in_=rs_bf[:, :, :])

            def attnv_ops(j, P, reverse=False):
                ops = []
                for tq in ([3, 2, 1, 0] if reverse else range(4)):
                    t = 4 * j + tq
                    ops.append(("alloc", t))
                    for i in range(t + 1):
                        ops.append(("mm", t, i))
                    ops.append(("evac", t))
                return ops

            def emit_attnv_op(op, P, state):
                if op[0] == "alloc":
                    t = op[1]
                    state[t] = ps_b.tile([128, CPC], f32, tag="vps", name=f"ops{t}")
                elif op[0] == "mm":
                    _, t, i = op
                    nc.tensor.matmul(
                        state[t][:, :],
                        P[i][:, 128 * (t % 4) : 128 * (t % 4) + 128],
                        v_sb[:, i, :],
                        start=(i == 0),
                        stop=(i == t),
                    )
                else:
                    t = op[1]
                    pair = t // 2
                    key = ("stage", pair)
                    if key not in state:
                        state[key] = opool.tile(
                            [128, 2, 512], bf16, tag="osb", name=f"osb{pair}"
                        )
                    st = state[key]
                    nc.scalar.copy(st[:, t % 2, :], state[t][:, :])
                    nfill = state.get(("nfill", pair), 0) + 1
                    state[("nfill", pair)] = nfill
                    if nfill == 2:
                        nc.sync.dma_start(out=out[pair, :, :], in_=st[:, :, :])

            # ---- V projection groups 1-3, with block 3's scores interleaved
            # into the tail so its exp chain (ACT) finishes before attn@V
            # needs P ----
            Ssum3 = [
                spool.tile([128, 512], f32, tag=f"S{c}", name=f"S3_{c}")
                for c in range(2)
            ]
            P3 = []
            for t in range(4, 16):
                v_ps = ps_b.tile([128, CPC], f32, tag="vps")
                for c in range(NCHUNK):
                    nc.tensor.matmul(
                        v_ps[:, :],
                        xt[:, c, 128 * t : 128 * (t + 1)],
                        wv[:, c, :],
                        start=(c == 0),
                        stop=(c == NCHUNK - 1),
                    )
                if t % 2 == 1:
                    nc.vector.tensor_copy(v_sb[:, t, :], v_ps[:, :])
                else:
                    nc.scalar.copy(v_sb[:, t, :], v_ps[:, :])
                if 8 <= t < 16:
                    P3.extend(emit_scores(3, 2 * (t - 8), Ssum3))
                    P3.extend(emit_scores(3, 2 * (t - 8) + 1, Ssum3))
            export_rsum(3, Ssum3)

            # ---- attention: blocks in reverse order, software-pipelined ----
            # Section s runs block j's scores/exp/rowsum while the previous
            # section's (larger) block does its attn@V - interleaved in PE
            # program order so neither phase stalls the in-order PE queue.
            prev = (3, P3)  # block 3 scored during vproj; attn@V pending
            for j in [2, 1, 0, None]:
                av = attnv_ops(*prev, reverse=(j is None)) if prev is not None else []
                avP = prev[1] if prev is not None else None
                av_state = {}
                if j is None:
                    for op in av:
                        emit_attnv_op(op, avP, av_state)
                    break
                n = 4 * j + 4
                Ssum = [
                    spool.tile([128, 512], f32, tag=f"S{c}", name=f"S{j}_{c}")
                    for c in range(2)
                ]
                P = []
                A = list(range(n))  # score emissions
                # front-load a couple of score pairs, then interleave the
                # previous block's attn@V ops
                front = min(3, len(A))
                k_av = 0
                for idx, i in enumerate(A):
                    P.extend(emit_scores(j, i, Ssum))
                    if idx >= front - 1:
                        want = (idx + 1 - front + 1) * len(av) / max(
                            1, len(A) - front + 1
                        )
                        while k_av < len(av) and k_av < want:
                            emit_attnv_op(av[k_av], avP, av_state)
                            k_av += 1
                while k_av < len(av):
                    emit_attnv_op(av[k_av], avP, av_state)
                    k_av += 1
                export_rsum(j, Ssum)
                prev = (j, P)

    nc.compile()
    return nc


def _get_nc():
    if "nc" not in _cache:
        _cache["nc"] = _build()
    return _cache["nc"]


def _prep_in_maps(x, WQ_w, WQ_b, WK_w, WK_b, WV_w, WV_b):
    bf = mybir.dt.np(bf16)
    wqk = np.concatenate([WQ_w, WK_w], axis=0)  # [128, D]
    wqkT = np.ascontiguousarray(
        wqk.T.reshape(NCHUNK, 128, 128).transpose(1, 0, 2)
    ).astype(bf)
    bqkr = np.concatenate([WQ_b, WK_b]).reshape(1, 128).astype(bf)

    # masks[kk, m, qq] = 1 if 128*m + kk <= qq else 0
    kk = np.arange(128)[:, None]
    qq = np.arange(512)[None, :]
    masks = np.ascontiguousarray(
        np.stack([(128 * m + kk <= qq) for m in range(4)], axis=0).transpose(1, 0, 2)
    ).astype(bf)
    in_maps = []
    for core in range(N_CORES):
        b, half = core // 2, core % 2
        xTb = np.ascontiguousarray(x[b].T).reshape(NCHUNK, 128, S).astype(bf)
        wv_sl = WV_w[half * CPC : (half + 1) * CPC]  # [CPC, D]
        wvT = np.ascontiguousarray(wv_sl.T).reshape(NCHUNK, 128, CPC).astype(bf)
        in_maps.append(
            {
                "xT": xTb,
                "wqkT": wqkT,
                "bqkr": bqkr,
                "wvT": wvT,
                "masks": masks,
            }
        )
    return in_maps


def _run(in_maps, trace=False, **kw):
    nc = _get_nc()
    return run_bass_kernel_spmd(
        nc, in_maps, core_ids=list(range(N_CORES)), trace=trace, **kw
    )


def kernel(x, WQ_w, WQ_b, WK_w, WK_b, WV_w, WV_b):
    x = np.asarray(x, dtype=np.float32)
    in_maps = _prep_in_maps(
        x,
        np.asarray(WQ_w, np.float32),
        np.asarray(WQ_b, np.float32),
        np.asarray(WK_w, np.float32),
        np.asarray(WK_b, np.float32),
        np.asarray(WV_w, np.float32),
        np.asarray(WV_b, np.float32),
    )
    res = _run(in_maps, trace=False)
    out = np.empty((B, S, D), dtype=np.float32)
    for core in range(N_CORES):
        b, half = core // 2, core % 2
        shard = (
            res.results[core]["out"]
            .astype(np.float32)
            .reshape(8, 128, 2, 512)
            .transpose(0, 2, 1, 3)
            .reshape(S, CPC)
        )
        if half == 0:
            rs = (
                res.results[core]["rsum"]
                .astype(np.float32)
                .sum(axis=(1, 2))
                .reshape(S)
            )
            out[b] = 0.0
        out[b, :, half * CPC : (half + 1) * CPC] = shard
        if half == 1:
            out[b] /= rs[:, None]
    out += np.asarray(WV_b, np.float32)[None, None, :]
    return out
